# revision 1
# baseline (speedup 1.0000x reference)
"""Multi-head attention on 8 Trainium2 NeuronCores.

Sharding: core c = (batch n, head-group g); n = c // 4, g = c % 4.
Each core computes attention for its 4 heads of its batch entry plus the
fc_out partial product for those heads' columns of Wo; the host sums the
4 partials per batch (and adds the bias) to unshard.

Per-core pipeline (all matmuls bf16, accumulation f32 in PSUM):
  A) qT/kT projections head-pair-stacked ([d,L] layout, pair in partition
     halves 0-63 / 64-127), v projection in [k, d] layout with a ones
     column appended per head (accumulates the softmax denominator for
     free during attn@v). Scores+exp for the first (pair, q-superchunk)
     are woven into the projection loops so ScalarE (the exp bottleneck,
     ~143us of work) starts as early as possible.
  B) scoresT in [k, q] layout (K=64 row-tiled pairs: both heads of a pair
     run concurrently in the PE array), exp on ScalarE straight out of
     PSUM (scale=1/32; no max subtraction needed: scores ~ N(0, 1/16)),
     attn@v accumulated over k tiles into [d+1, q] PSUM (row 64 =
     denominator). Per-(pair,qs) normalization is inlined: reciprocal of
     the 4 denominator rows, DRAM-bounce partition-broadcast, multiply.
     Odd heads are DMA-shifted to partitions 64-127 to form K=128 pairs
     for fc.
  C) fc_out partial = WoPair.T @ outTP with K=128 head pairs; PSUM
     evacuations alternate ScalarE/VectorE; bias is applied on the host
     while summing the partials.
"""

import contextlib as _contextlib
import os
import sys

for _p in ("/opt/trn_rl_repo",):
    if _p not in sys.path and os.path.isdir(_p):
        sys.path.insert(0, _p)

import numpy as np
import ml_dtypes

import concourse.bass as bass
import concourse.mybir as mybir
import concourse.tile as tile
from concourse import bacc
from concourse.bass import ds, ts
from concourse.bass_utils import run_bass_kernel_spmd

BF16 = ml_dtypes.bfloat16
F32 = np.float32

EMBED = 1024
HEADS = 16
HD = 64  # head dim
NB = 2  # batch
L = 2048  # sequence length
NCORES = 8
HPG = 4  # heads per core (group)
NPAIRS = 2  # head pairs per core
ET = EMBED // 128  # 8 contraction tiles for projections
LT = L // 128  # 16 k tiles
QS = 1024  # q superchunk (exp free-dim)
NQS = L // QS  # 2
NLC = L // 512  # 4 512-wide l chunks

N_EARLY = 32  # early-emitted scores/exp steps; MUST be <= expp bufs

SCALE = 1.0 / np.sqrt(np.float32(EMBED))  # 1/32

LAST_EXEC_TIME_NS = None
LAST_RESULTS = None

_nc_cache = None


def build_nc():
    """Build + compile the per-core Bass program (same program on all cores)."""
    nc = bacc.Bacc("TRN2")
    f32 = mybir.dt.float32
    bf16 = mybir.dt.bfloat16
    EXP = mybir.ActivationFunctionType.Exp

    xT_d = nc.declare_dram_parameter("xT", [EMBED, L], bf16, isOutput=False)
    # weights arrive partition-major from the host so every DMA row is a
    # contiguous 2-8KB run (256B rows serialize the DGE and gate startup)
    wqk_d = nc.declare_dram_parameter("wqk", [128, 4, ET, 128], bf16, isOutput=False)
    wv_d = nc.declare_dram_parameter("wv", [128, ET, HPG * HD], bf16, isOutput=False)
    wo_d = nc.declare_dram_parameter("wo", [128, NPAIRS, ET, 128], bf16, isOutput=False)
    out_d = nc.declare_dram_parameter("out", [EMBED, L], bf16, isOutput=True)
    recip_dram = nc.dram_tensor("recip_dram", [16, 512], bf16)

    with tile.TileContext(nc) as tc:
        with (
            tc.tile_pool(name="expp", bufs=N_EARLY) as expp,
            tc.tile_pool(name="singles", bufs=1) as singles,
            tc.tile_pool(name="drowp", bufs=3) as drowp,
            tc.tile_pool(name="rbp", bufs=8) as rbp,
            tc.tile_pool(name="shiftp", bufs=3) as shiftp,
            tc.tile_pool(name="outp", bufs=3) as outp,
        ):
            # ---- resident SBUF tensors ----
            xT_sb = singles.tile([128, ET, L], bf16, name="xT_sb")
            wqk_sb = singles.tile([128, 4, ET, 128], bf16, name="wqk_sb")
            wv_sb = singles.tile([128, ET, HPG * HD], bf16, name="wv_sb")
            wo_sb = singles.tile([128, NPAIRS, ET, 128], bf16, name="wo_sb")
            qt_sb = singles.tile([128, NPAIRS, L], bf16, name="qt_sb")
            kt_sb = singles.tile([128, NPAIRS, L], bf16, name="kt_sb")
            v_sb = singles.tile([128, LT, HPG, HD + 1], bf16, name="v_sb")
            outTP_sb = singles.tile([128, NPAIRS, L], bf16, name="outTP_sb")
            num_sb = singles.tile([HD, HPG, L], bf16, name="num_sb")
            # per-(pair,qs) denominator blocks: 4 rows each, base partition 0
            denom_bl = [
                singles.tile([4, 512], f32, name=f"denom{b}") for b in range(4)
            ]
            recip_bl = [
                singles.tile([4, 512], f32, name=f"recip{b}") for b in range(4)
            ]
            recipb_bl = [
                singles.tile([4, 512], bf16, name=f"recipb{b}") for b in range(4)
            ]

            # ---- input DMAs, ordered so compute starts early ----
            xT_ap = xT_d[:].rearrange("(t p) l -> p t l", p=128)
            # ONE queue, strict priority order: the feed runs ~215GB/s
            # total no matter how many queues, so ordering is everything.
            # xT streams column-half-major: the first scores need only
            # wqk01 + the first 2MB (q cols 0-1023, k tiles 0-7).
            for j in range(2):
                nc.sync.dma_start(out=wqk_sb[:, j, :, :], in_=wqk_d[:, j, :, :])
            for et in range(ET):
                nc.sync.dma_start(
                    out=xT_sb[:, et, ts(0, 1024)], in_=xT_ap[:, et, ts(0, 1024)]
                )
            nc.sync.dma_start(out=wv_sb, in_=wv_d[:])
            for et in range(ET):
                nc.sync.dma_start(
                    out=xT_sb[:, et, ts(1, 1024)], in_=xT_ap[:, et, ts(1, 1024)]
                )
            for j in range(2, 4):
                nc.sync.dma_start(out=wqk_sb[:, j, :, :], in_=wqk_d[:, j, :, :])
            nc.sync.dma_start(out=wo_sb, in_=wo_d[:])

            # scores PSUM pool spans phases A+B only; closed before fc so
            # its banks are free for psC
            _psS_stack = _contextlib.ExitStack()
            psS = _psS_stack.enter_context(
                tc.tile_pool(name="psS", bufs=2, space="PSUM")
            )

            _psAV_stack = _contextlib.ExitStack()
            _psAV3_stack = _contextlib.ExitStack()
            psAV3 = None

            ex_store = {}  # (pair, qs, side, k) -> exp tile emitted early
            sc_emitted = set()
            av_tiles = {}
            av_done = set()

            def get_av(pair, qs, side):
                key = (pair, qs, side)
                pool = psAV if key == (0, 0, 0) else psAV3
                if key not in av_tiles:
                    av_tiles[key] = [
                        pool.tile(
                            [128, 512],
                            f32,
                            tag="av",
                            name=f"av{pair}{qs}{side}{h}",
                        )
                        for h in range(2)
                    ]
                return av_tiles[key]

            _weave_iter = iter(range(LT))

            def emit_av_weave():
                # per j2/j3 step: two attn@v k-tiles for (pair0,qs0,side0)
                # plus the matching look-ahead scores/exp for (pair0,qs1,
                # side0) - pops and pushes balance so the exp pool stays
                # exactly full and ScalarE never drains
                for _ in range(2):
                    k = next(_weave_iter, None)
                    if k is None:
                        return
                    av = get_av(0, 0, 0)
                    ex = ex_store.pop((0, 0, 0, k))
                    for half in range(2):
                        nc.tensor.matmul(
                            av[half][0 : HD + 1, :],
                            v_sb[:, k, 0, :],
                            ex[:, ts(half, 512)],
                            start=(k == 0),
                            stop=(k == LT - 1),
                        )
                    lkey = (0, 1, 0, k)
                    sc_emitted.add(lkey)
                    ex_store[lkey] = emit_sc_exp(*lkey)
                    if k == LT - 1:
                        av_done.add((0, 0, 0))

            def emit_sc_exp(pair, qs, side, k):
                base = side * HD
                sc = psS.tile([128, QS], f32, tag="sc", name=f"sc{side}")
                for half in range(2):
                    nc.tensor.matmul(
                        sc[:, ts(half, 512)],
                        kt_sb[base : base + HD, pair, ts(k, 128)],
                        qt_sb[base : base + HD, pair, ds(qs * QS + half * 512, 512)],
                        start=True,
                        stop=True,
                    )
                ex = expp.tile([128, QS], bf16, tag="exp", name="ex")
                nc.scalar.activation(ex, sc, EXP, scale=float(SCALE))
                return ex

            # early-emit list: scores+exp for (pair0, qs0) woven into the
            # v-projection and j2/j3 loops so ScalarE starts early.
            # Capped at the exp pool size: an early exp whose slot reuse
            # depends on a phase-B attn@v consumer would deadlock the PE
            # FIFO (attn@v sits behind phase-A matmuls).
            early = [(0, 0, s, k) for k in range(LT) for s in range(2)]
            early = early[:N_EARLY]

            def emit_sc_exp_pair(pair, qs, k):
                # both sides of a head pair, matmuls interleaved so the
                # (0,0) and (64,0) row-group tiles overlap in the PE array
                scs = [
                    psS.tile([128, QS], f32, tag="sc", name=f"sc{s}")
                    for s in range(2)
                ]
                for half in range(2):
                    for side in range(2):
                        base = side * HD
                        nc.tensor.matmul(
                            scs[side][:, ts(half, 512)],
                            kt_sb[base : base + HD, pair, ts(k, 128)],
                            qt_sb[
                                base : base + HD,
                                pair,
                                ds(qs * QS + half * 512, 512),
                            ],
                            start=True,
                            stop=True,
                        )
                out = []
                for side in range(2):
                    ex = expp.tile([128, QS], bf16, tag="exp", name="ex")
                    nc.scalar.activation(ex, scs[side], EXP, scale=float(SCALE))
                    out.append(ex)
                return out

            def emit_early():
                if len(early) >= 2 and early[0][:3] == (0, 0, 0):
                    k = early[0][3]
                    if early[1] == (0, 0, 1, k):
                        k0, k1 = early.pop(0), early.pop(0)
                        exs = emit_sc_exp_pair(0, 0, k)
                        sc_emitted.add(k0)
                        sc_emitted.add(k1)
                        ex_store[k0], ex_store[k1] = exs[0], exs[1]
                        return
                if early:
                    key = early.pop(0)
                    sc_emitted.add(key)
                    ex_store[key] = emit_sc_exp(*key)

            # ================= Phase A: projections =================
            # j0+j1 computed together in two column-half passes, matching
            # the xT DMA stream: pass lh covers q/k columns lh*1024..+1024
            # for both j, so the first scores only wait on the first 2MB
            # of xT.  4 accumulator banks + psS 4 = 8.
            def emit_warm(n):
                # junk matmuls on already-resident weights: keep the PE
                # busy through the DMA-paced stretches so it ramps to (and
                # holds) the full 2.4GHz p-state
                warm = psS.tile([128, QS], f32, tag="sc", name="warm")
                for _ in range(n):
                    nc.tensor.matmul(
                        warm[:, 0:512],
                        wqk_sb[:, 0, 0, :],
                        wqk_sb[:, 0, 0:4, :],
                        start=True,
                        stop=True,
                    )

            with tc.tile_pool(name="psA4", bufs=4, space="PSUM") as psA4:
                for lh in range(2):
                    pst = [
                        psA4.tile([128, 512], f32, tag="ps", name=f"qk{lh}_{i}")
                        for i in range(4)
                    ]
                    for et in range(ET):
                        for j in range(2):
                            for l2 in range(2):
                                nc.tensor.matmul(
                                    pst[j * 2 + l2],
                                    wqk_sb[:, j, et, :],
                                    xT_sb[:, et, ts(lh * 2 + l2, 512)],
                                    start=(et == 0),
                                    stop=(et == ET - 1),
                                )
                    for j in range(2):
                        dst = qt_sb if j == 0 else kt_sb
                        for l2 in range(2):
                            nc.vector.tensor_copy(
                                dst[:, 0, ts(lh * 2 + l2, 512)],
                                pst[j * 2 + l2],
                            )

            psAV = _psAV_stack.enter_context(
                tc.tile_pool(name="psAV", bufs=2, space="PSUM")
            )
            with tc.tile_pool(name="psA2", bufs=2, space="PSUM") as psA2:
                # v in [k, d] layout, 4 heads side by side
                for lt in range(LT):
                    emit_early()
                    pv = psA2.tile([128, 512], f32, tag="ps2", name=f"v{lt}")
                    pv = pv[:, : HPG * HD]
                    for et in range(ET):
                        nc.tensor.matmul(
                            pv,
                            xT_sb[:, et, ts(lt, 128)],
                            wv_sb[:, et, :],
                            start=(et == 0),
                            stop=(et == ET - 1),
                        )
                    nc.vector.tensor_copy(
                        v_sb[:, lt, :, 0:HD],
                        pv.rearrange("p (h d) -> p h d", h=HPG),
                    )
                    emit_early()
                nc.vector.memset(v_sb[:, :, :, HD : HD + 1], 1.0)
                for j in range(2, 4):
                    dst = qt_sb if j == 2 else kt_sb
                    for lc in range(NLC):
                        pst = psA2.tile(
                            [128, 512], f32, tag="ps2", name=f"qk{j}_{lc}"
                        )
                        for et in range(ET):
                            nc.tensor.matmul(
                                pst,
                                wqk_sb[:, j, et, :],
                                xT_sb[:, et, ts(lc, 512)],
                                start=(et == 0),
                                stop=(et == ET - 1),
                            )
                        nc.vector.tensor_copy(dst[:, 1, ts(lc, 512)], pst)
                        emit_av_weave()

            # ================= Phase B: attention =================
            # Uniform pipeline over 8 side-blocks (pair, qs, side). During
            # side-block i's attn@v k-loop we emit scores/exp for
            # side-block i+2, so ScalarE (the 143us exp bottleneck) keeps
            # streaming and the 32-slot exp pool stays exactly full.
            pending_norms = []

            def emit_norm_muls(pair, qs, pend):
                for rb, side2, half in pend:
                    h2 = pair * 2 + side2
                    col0 = qs * QS + half * 512
                    if side2 == 0:
                        nc.vector.tensor_mul(
                            outTP_sb[0:HD, pair, ds(col0, 512)],
                            num_sb[0:HD, h2, ds(col0, 512)],
                            rb,
                        )
                    else:
                        # odd head: normalize into a temp, then DMA-shift
                        # to partitions 64-127
                        tmp = shiftp.tile([HD, 512], bf16, tag="sh", name="sh")
                        nc.vector.tensor_mul(
                            tmp, num_sb[0:HD, h2, ds(col0, 512)], rb
                        )
                        nc.sync.dma_start(
                            out=outTP_sb[HD:128, pair, ds(col0, 512)], in_=tmp
                        )

            side_blocks = [
                (p, q, s) for p in range(NPAIRS) for q in range(NQS)
                for s in range(2)
            ]
            ridx = 0
            rbase = {}
            for i, (pair, qs, side) in enumerate(side_blocks):
                blk = pair * NQS + qs
                if side == 0:
                    rbase[blk] = ridx
                h_local = pair * 2 + side
                av = get_av(pair, qs, side)
                if (pair, qs, side) not in av_done:
                    for k in range(LT):
                        if i + 2 < len(side_blocks):
                            lkey = side_blocks[i + 2] + (k,)
                            if lkey not in sc_emitted:
                                sc_emitted.add(lkey)
                                ex_store[lkey] = emit_sc_exp(*lkey)
                        ex = ex_store.pop((pair, qs, side, k))
                        for half in range(2):
                            nc.tensor.matmul(
                                av[half][0 : HD + 1, :],
                                v_sb[:, k, h_local, :],
                                ex[:, ts(half, 512)],
                                start=(k == 0),
                                stop=(k == LT - 1),
                            )
                # evacuate this head's numerators + denominator rows
                for half in range(2):
                    avt = av[half]
                    col0 = qs * QS + half * 512
                    nc.vector.tensor_copy(
                        num_sb[:, h_local, ds(col0, 512)], avt[0:HD, :]
                    )
                    dr = drowp.tile([HD + 1, 512], f32, tag="dr", name="dr")
                    nc.vector.tensor_copy(
                        dr[HD : HD + 1, :], avt[HD : HD + 1, :]
                    )
                    nc.sync.dma_start(
                        out=denom_bl[blk][ridx - rbase[blk] : ridx - rbase[blk] + 1, :],
                        in_=dr[HD : HD + 1, :],
                    )
                    ridx += 1
                if (pair, qs, side) == (0, 0, 0) and psAV3 is None:
                    _psAV_stack.close()
                    psAV3 = _psAV3_stack.enter_context(
                        tc.tile_pool(name="psAV3", bufs=3, space="PSUM")
                    )
                if side != 1:
                    continue
                # normalize this (pair, qs): reciprocal -> bf16 -> DRAM
                # bounce broadcast issued now; the multiplies are deferred
                # one block so the DVE never stalls on the DMA roundtrip
                r0 = rbase[blk]
                nc.vector.reciprocal_approx_fast(recip_bl[blk], denom_bl[blk])
                nc.vector.tensor_copy(recipb_bl[blk], recip_bl[blk])
                nc.sync.dma_start(out=recip_dram[r0:ridx, :], in_=recipb_bl[blk])
                pend = []
                j = r0
                for side2 in range(2):
                    for half in range(2):
                        rb = rbp.tile([HD, 512], bf16, tag="rb", name="rb")
                        nc.sync.dma_start(
                            out=rb,
                            in_=recip_dram[j : j + 1, :].to_broadcast([HD, 512]),
                        )
                        pend.append((rb, side2, half))
                        j += 1
                pending_norms.append((pair, qs, pend))
                if len(pending_norms) > 1:
                    emit_norm_muls(*pending_norms.pop(0))
            while pending_norms:
                emit_norm_muls(*pending_norms.pop(0))
            _psAV3_stack.close()

            # warm-keeper: dense dummy matmuls carry the PE through the
            # final normalize window so fc starts at full clock (HAM
            # re-throttles after ~3.4us of PE idle)
            warm = psS.tile([128, 512], f32, tag="sc", name="warm")
            for _ in range(24):
                nc.tensor.matmul(
                    warm,
                    wo_sb[:, 0, 0, :],
                    outTP_sb[:, 0, 0:512],
                    start=True,
                    stop=True,
                )

            _psS_stack.close()  # free scores banks before fc

            # ================= Phase C: fc_out partial =================
            # bias is applied on the host during unsharding; evacuations
            # alternate ScalarE/VectorE in 1024-wide chunks to shorten the
            # drain chain after the last matmul
            out_ap = out_d[:].rearrange("(t p) l -> p t l", p=128)
            with tc.tile_pool(name="psC", bufs=4, space="PSUM") as psC:
                for lcp in range(2):
                    for et in range(ET):
                        fps = psC.tile(
                            [128, 1024], f32, tag="fc", name=f"fc{et}_{lcp}"
                        )
                        for half in range(2):
                            for pair in range(NPAIRS):
                                nc.tensor.matmul(
                                    fps[:, ts(half, 512)],
                                    wo_sb[:, pair, et, :],
                                    outTP_sb[
                                        :, pair, ds(lcp * 1024 + half * 512, 512)
                                    ],
                                    start=(pair == 0),
                                    stop=(pair == NPAIRS - 1),
                                )
                        # both halves stage into xT_sb (dead after phase
                        # A) and ship as 1MB DMAs: per-chunk DMAs cost ~1us
                        # of serialized descriptor generation each
                        ob = xT_sb[:, et, ts(lcp, 1024)]
                        if et % 2 == 0:
                            nc.scalar.copy(ob, fps)
                        else:
                            nc.vector.tensor_copy(ob, fps)
                        if et % 2 == 1:
                            # 512KB ships: the final piece is the serial
                            # tail, so smaller pieces drain sooner; earlier
                            # ships overlap fc compute
                            eh = et // 2
                            nc.sync.dma_start(
                                out=out_ap[:, ts(eh, 2), ts(lcp, 1024)],
                                in_=xT_sb[:, ts(eh, 2), ts(lcp, 1024)],
                            )

    nc.compile()
    return nc


def get_nc():
    global _nc_cache
    if _nc_cache is None:
        _nc_cache = build_nc()
    return _nc_cache


def make_core_inputs(x, Wq, Wk, Wv, Wo, bo):
    """Build the 8 per-core input maps from the full-size inputs."""
    x = np.asarray(x, F32)
    Wq = np.asarray(Wq, F32)
    Wk = np.asarray(Wk, F32)
    Wv = np.asarray(Wv, F32)
    Wo = np.asarray(Wo, F32)
    bo = np.asarray(bo, F32)

    xT_b = [np.ascontiguousarray(x[n].T).astype(BF16) for n in range(NB)]

    in_maps = []
    for c in range(NCORES):
        n, g = divmod(c, HPG)
        heads = [g * HPG + i for i in range(HPG)]

        wqk = np.empty((4, EMBED, 128), F32)
        for j in range(4):
            pair, qk = divmod(j, 2)
            hA = heads[2 * pair]
            hB = heads[2 * pair + 1]
            W = Wq if qk == 0 else Wk
            wqk[j, :, 0:HD] = W[hA * HD : (hA + 1) * HD, :].T
            wqk[j, :, HD:128] = W[hB * HD : (hB + 1) * HD, :].T

        wv = np.concatenate(
            [Wv[h * HD : (h + 1) * HD, :].T for h in heads], axis=1
        )  # [1024, 256]

        wo = np.empty((NPAIRS, ET, 128, 128), F32)
        for pair in range(NPAIRS):
            hA = heads[2 * pair]
            hB = heads[2 * pair + 1]
            for et in range(ET):
                blk = Wo[et * 128 : (et + 1) * 128, :]
                wo[pair, et, 0:HD, :] = blk[:, hA * HD : (hA + 1) * HD].T
                wo[pair, et, HD:128, :] = blk[:, hB * HD : (hB + 1) * HD].T

        # partition-major relayouts: [p, ...] with contiguous per-p rows
        wqk_t = np.ascontiguousarray(
            wqk.reshape(4, ET, 128, 128).transpose(2, 0, 1, 3)
        )
        wv_t = np.ascontiguousarray(
            wv.reshape(ET, 128, HPG * HD).transpose(1, 0, 2)
        )
        wo_t = np.ascontiguousarray(wo.transpose(2, 0, 1, 3))

        in_maps.append(
            {
                "xT": xT_b[n],
                "wqk": wqk_t.astype(BF16),
                "wv": wv_t.astype(BF16),
                "wo": wo_t.astype(BF16),
            }
        )
    return in_maps


def combine_outputs(results, bo):
    """Sum the per-core fc_out partials, add bias, transpose to [N, L, E]."""
    out = np.empty((NB, L, EMBED), F32)
    for n in range(NB):
        acc = results[n * HPG]["out"].astype(F32)
        for g in range(1, HPG):
            acc = acc + results[n * HPG + g]["out"].astype(F32)
        out[n] = acc.T + np.asarray(bo, F32)
    return out


def kernel(x, Wq, Wk, Wv, Wo, bo):
    global LAST_EXEC_TIME_NS, LAST_RESULTS
    nc = get_nc()
    in_maps = make_core_inputs(x, Wq, Wk, Wv, Wo, bo)
    trace = bool(os.environ.get("KERNEL_TRACE"))
    kw = {}
    if trace:
        kw["trace"] = True
        kw["trace_cores"] = list(range(NCORES))
    res = run_bass_kernel_spmd(nc, in_maps, list(range(NCORES)), **kw)
    LAST_EXEC_TIME_NS = res.exec_time_ns
    LAST_RESULTS = res
    return combine_outputs(res.results, bo)



# revision 7
# speedup vs baseline: 1.1010x; 1.1010x over previous
"""Multi-head attention on 8 Trainium2 NeuronCores.

Sharding: core c = (batch n, head-group g); n = c // 4, g = c % 4.
Each core computes attention for its 4 heads of its batch entry plus the
fc_out partial product for those heads' columns of Wo; the host sums the
4 partials per batch (and adds the bias) to unshard.

Pipeline (all matmuls bf16, f32 PSUM accumulation). ScalarE exp is the
roofline (~147us: 128 x [128,1024] activations at (N+352)/1.2 ns); the
whole schedule is built to start that stream early and never starve it:

  A) Minimal head start: pair-0 q/k projection for q-columns 0-1023
     (lh0) only, then the exp-paced main loop begins. Everything else
     (lh1, pair-1 q/k, v, fc lcp=0) is emitted as filler batches
     between steps.
  B) 64 steps (4 blocks (pair,qs) x 16 k-tiles). Per step: scores for
     step s+1 (2 sides x 2 halves, [64,128] row-tiled), exp A + exp B
     ([128,1024] each, psS bufs=2 gives a side-staggered double
     buffer: A's banks recycle under B's exp), then attn@v for step s:
     col-tiled pairs - head A -> PSUM partitions 0-63, head B -> 64-127
     run concurrently (M=64 each), landing directly in the outTP
     partition layout (no post-hoc shift DMA). The softmax denominator
     comes from DVE partial sums of the exp tiles (bf16 tensor_tensor,
     2x mode) reduced across partitions by a col-tiled ones-matmul
     whose M=64 replication IS the broadcast for the normalize multiply
     (no DRAM round-trips).
  C) fc_out: lcp=0 chunks run as fillers once qs=0 blocks are
     normalized; lcp=1 is the tail, evacuations alternating
     ScalarE/VectorE, output shipped in 512KB pieces.
"""

import contextlib as _contextlib
import os
import sys

for _p in ("/opt/trn_rl_repo",):
    if _p not in sys.path and os.path.isdir(_p):
        sys.path.insert(0, _p)

import numpy as np
import ml_dtypes

import concourse.bass as bass
import concourse.mybir as mybir
import concourse.tile as tile
from concourse import bacc
from concourse.bass import ds, ts
from concourse.bass_utils import run_bass_kernel_spmd

BF16 = ml_dtypes.bfloat16
F32 = np.float32

EMBED = 1024
HEADS = 16
HD = 64  # head dim
NB = 2  # batch
L = 2048  # sequence length
NCORES = 8
HPG = 4  # heads per core (group)
NPAIRS = 2  # head pairs per core
ET = EMBED // 128  # 8 contraction tiles for projections
LT = L // 128  # 16 k tiles

SCALE = 1.0 / np.sqrt(np.float32(EMBED))  # 1/32

# block order: qs=0 blocks first so fc lcp=0 can run as filler work
BLOCKS = [(0, 0), (1, 0), (0, 1), (1, 1)]  # (pair, qs)
NSTEPS = len(BLOCKS) * LT  # 64

LAST_EXEC_TIME_NS = None
LAST_RESULTS = None

_nc_cache = None


def build_nc():
    """Build + compile the per-core Bass program (same program on all cores)."""
    nc = bacc.Bacc("TRN2")
    f32 = mybir.dt.float32
    bf16 = mybir.dt.bfloat16
    EXP = mybir.ActivationFunctionType.Exp

    xT_d = nc.declare_dram_parameter("xT", [EMBED, L], bf16, isOutput=False)
    # weights arrive partition-major from the host so every DMA row is a
    # contiguous 2-8KB run (256B rows serialize the DGE and gate startup)
    wqk_d = nc.declare_dram_parameter("wqk", [128, 4, ET, 128], bf16, isOutput=False)
    wv_d = nc.declare_dram_parameter("wv", [128, ET, HPG * HD], bf16, isOutput=False)
    wo_d = nc.declare_dram_parameter("wo", [128, NPAIRS, ET, 128], bf16, isOutput=False)
    out_d = nc.declare_dram_parameter("out", [EMBED, L], bf16, isOutput=True)

    with tile.TileContext(nc) as tc:
        with (
            tc.tile_pool(name="singles", bufs=1) as singles,
            tc.tile_pool(name="expp", bufs=10) as expp,
            tc.tile_pool(name="pap", bufs=2) as pap,
            tc.tile_pool(name="rcp", bufs=3) as rcp,
        ):
            # ---- resident SBUF tensors ----
            xT_sb = singles.tile([128, ET, L], bf16, name="xT_sb")
            wqk_sb = singles.tile([128, 4, ET, 128], bf16, name="wqk_sb")
            wv_sb = singles.tile([128, ET, HPG * HD], bf16, name="wv_sb")
            wo_sb = singles.tile([128, NPAIRS, ET, 128], bf16, name="wo_sb")
            qt_sb = singles.tile([128, NPAIRS, L], bf16, name="qt_sb")
            kt_sb = singles.tile([128, NPAIRS, L], bf16, name="kt_sb")
            v_sb = singles.tile([128, LT, HPG, HD], bf16, name="v_sb")
            outTP_sb = singles.tile([128, NPAIRS, L], bf16, name="outTP_sb")
            ones_sb = singles.tile([128, HD], bf16, name="ones_sb")
            warm_in = singles.tile([1, 2], bf16, name="warm_in")
            warm_out = singles.tile([1, 2], bf16, name="warm_out")

            # ---- input DMAs, ONE queue, strict priority order ----
            xT_ap = xT_d[:].rearrange("(t p) l -> p t l", p=128)
            for j in range(2):
                nc.sync.dma_start(out=wqk_sb[:, j, :, :], in_=wqk_d[:, j, :, :])
            for et in range(ET):
                nc.sync.dma_start(
                    out=xT_sb[:, et, ts(0, 1024)], in_=xT_ap[:, et, ts(0, 1024)]
                )
            nc.sync.dma_start(out=wv_sb, in_=wv_d[:])
            for et in range(ET):
                nc.sync.dma_start(
                    out=xT_sb[:, et, ts(1, 1024)], in_=xT_ap[:, et, ts(1, 1024)]
                )
            for j in range(2, 4):
                nc.sync.dma_start(out=wqk_sb[:, j, :, :], in_=wqk_d[:, j, :, :])
            nc.sync.dma_start(out=wo_sb, in_=wo_d[:])

            nc.vector.memset(ones_sb, 1.0)
            nc.vector.memset(warm_in, 1.0)
            # preload the exp spline tables during the DMA-bound window
            nc.scalar.activation(warm_out, warm_in, EXP)

            # scores PSUM: two [128,1024] f32 bufs (4 banks). The bufs=2
            # rotation is the side-staggered double buffer: side A of
            # step s+1 allocates as soon as exp(A_s) drains, while
            # exp(B_s) still runs.
            _psS_stack = _contextlib.ExitStack()
            psS = _psS_stack.enter_context(
                tc.tile_pool(name="psS", bufs=2, space="PSUM")
            )

            # ---- warm PE through the DMA-gated start ----
            warm = psS.tile([128, 512], f32, tag="sc", name="warm0")
            for _ in range(16):
                nc.tensor.matmul(
                    warm,
                    wqk_sb[:, 0, 0, :],
                    wqk_sb[:, 0, 0:4, :],
                    start=True,
                    stop=True,
                )

            # ================= lh0: pair-0 q/k, columns 0-1023 ========
            with tc.tile_pool(name="psA4", bufs=4, space="PSUM") as psA4:
                pst = [
                    psA4.tile([128, 512], f32, tag="ps", name=f"qk0_{i}")
                    for i in range(4)
                ]
                for et in range(ET):
                    for j in range(2):
                        for l2 in range(2):
                            nc.tensor.matmul(
                                pst[j * 2 + l2],
                                wqk_sb[:, j, et, :],
                                xT_sb[:, et, ts(l2, 512)],
                                start=(et == 0),
                                stop=(et == ET - 1),
                            )
                for j in range(2):
                    dst = qt_sb if j == 0 else kt_sb
                    for l2 in range(2):
                        nc.vector.tensor_copy(
                            dst[:, 0, ts(l2, 512)], pst[j * 2 + l2]
                        )

            # filler psum pool (2 banks) for lh1/j23/v/fc-lcp0 chunks
            _psA2_stack = _contextlib.ExitStack()
            psA2 = _psA2_stack.enter_context(
                tc.tile_pool(name="psA2", bufs=2, space="PSUM")
            )

            # ---- filler batches: ~0.9-1.9us of PE work each ----------
            fillers = []

            def f_vchunk(lt):
                def go():
                    pv = psA2.tile([128, 512], f32, tag="ps2", name=f"v{lt}")
                    pv = pv[:, : HPG * HD]
                    for et in range(ET):
                        nc.tensor.matmul(
                            pv,
                            xT_sb[:, et, ts(lt, 128)],
                            wv_sb[:, et, :],
                            start=(et == 0),
                            stop=(et == ET - 1),
                        )
                    nc.vector.tensor_copy(
                        v_sb[:, lt, :, :],
                        pv.rearrange("p (h d) -> p h d", h=HPG),
                    )
                return go

            def f_qkchunk(j, lc):
                def go():
                    pq = psA2.tile([128, 512], f32, tag="ps2", name=f"qk{j}_{lc}")
                    for et in range(ET):
                        nc.tensor.matmul(
                            pq,
                            wqk_sb[:, j, et, :],
                            xT_sb[:, et, ts(lc, 512)],
                            start=(et == 0),
                            stop=(et == ET - 1),
                        )
                    dst = qt_sb if j % 2 == 0 else kt_sb
                    nc.vector.tensor_copy(dst[:, j // 2, ts(lc, 512)], pq)
                return go

            out_ap = out_d[:].rearrange("(t p) l -> p t l", p=128)
            fc_done = [[False] * 2 for _ in range(ET)]  # [et][lcp]

            def f_fcchunk(lcp, et, half, evac_engine):
                def go():
                    fp = psA2.tile([128, 512], f32, tag="ps2", name=f"fc{et}{half}")
                    for pair in range(NPAIRS):
                        nc.tensor.matmul(
                            fp,
                            wo_sb[:, pair, et, :],
                            outTP_sb[:, pair, ds(lcp * 1024 + half * 512, 512)],
                            start=(pair == 0),
                            stop=(pair == NPAIRS - 1),
                        )
                    ob = xT_sb[:, et, ds(lcp * 1024 + half * 512, 512)]
                    if evac_engine == "s":
                        nc.scalar.copy(ob, fp)
                    else:
                        nc.vector.tensor_copy(ob, fp)
                    fc_done[et][lcp] = half == 1
                    if half == 1 and et % 2 == 1 and fc_done[et - 1][lcp]:
                        eh = et // 2
                        nc.sync.dma_start(
                            out=out_ap[:, ts(eh, 2), ts(lcp, 1024)],
                            in_=xT_sb[:, ts(eh, 2), ts(lcp, 1024)],
                        )
                return go

            # (due_step, fn): popped at the top of each step.  Dues
            # respect both DMA arrival (step s runs at ~13+2.3s us;
            # wv ~14.5, xT second half ~16-24, wqk23 ~26) and consumer
            # deadlines (v[k] before block-0 av step k, lh1 before
            # scores k=8, pair-1 q/k before block-1 scores at step 15+).
            for lt in range(8):
                fillers.append((lt // 2, f_vchunk(lt)))
            fillers.append((4, f_qkchunk(0, 2)))
            fillers.append((4, f_qkchunk(1, 2)))
            fillers.append((5, f_qkchunk(0, 3)))
            fillers.append((5, f_qkchunk(1, 3)))
            for lt in range(8, LT):
                fillers.append((lt - 2, f_vchunk(lt)))
            fillers.append((9, f_qkchunk(2, 0)))
            fillers.append((10, f_qkchunk(2, 1)))
            fillers.append((11, f_qkchunk(3, 0)))
            fillers.append((13, f_qkchunk(3, 1)))
            fillers.append((15, f_qkchunk(3, 2)))
            fillers.append((17, f_qkchunk(3, 3)))
            fillers.append((19, f_qkchunk(2, 2)))
            fillers.append((21, f_qkchunk(2, 3)))

            def pop_due(s):
                while fillers and fillers[0][0] <= s:
                    fillers.pop(0)[1]()

            # ================= main exp-paced loop ====================
            # attn@v psum: 2 banks, single-buffered per block (the next
            # block's start=True write waits on this block's normalize,
            # ~1us at each boundary; av is not on ScalarE's critical
            # path).  Denominator tiles borrow the filler pool psA2.
            _psB_stack = _contextlib.ExitStack()
            psB = _psB_stack.enter_context(
                tc.tile_pool(name="psB", bufs=2, space="PSUM")
            )

            def emit_scores(s):
                blk, k = divmod(s, LT)
                pair, qs = BLOCKS[blk]
                sides = []
                for side in range(2):
                    st = psS.tile([128, 1024], f32, tag="sc", name=f"sc{side}")
                    base = side * HD
                    for half in range(2):
                        nc.tensor.matmul(
                            st[:, ts(half, 512)],
                            kt_sb[base : base + HD, pair, ts(k, 128)],
                            qt_sb[
                                base : base + HD,
                                pair,
                                ds(qs * 1024 + half * 512, 512),
                            ],
                            start=True,
                            stop=True,
                        )
                    sides.append(st)
                return sides

            def emit_exp(sides):
                ex = expp.tile([128, 2048], bf16, tag="exp", name="ex")
                for side in range(2):
                    nc.scalar.activation(
                        ex[:, ds(side * 1024, 1024)],
                        sides[side],
                        EXP,
                        scale=float(SCALE),
                    )
                return ex

            av_cur = [None]  # [av_h0, av_h1] for the in-flight block
            pa_cur = [None]  # partial-sum accumulator for the block

            def emit_av(s, ex):
                blk, k = divmod(s, LT)
                pair, qs = BLOCKS[blk]
                if k == 0:
                    av_cur[0] = [
                        psB.tile([128, 512], f32, tag="av", name=f"av{blk}{h}")
                        for h in range(2)
                    ]
                av = av_cur[0]
                # col-tiled pairs: head A -> partitions 0-63 (tile (0,0)),
                # head B -> 64-127 (tile (0,64)); interleave sides so the
                # two streams run concurrently
                for half in range(2):
                    for side in range(2):
                        nc.tensor.matmul(
                            av[half][side * HD : (side + 1) * HD, :],
                            v_sb[:, k, pair * 2 + side, :],
                            ex[:, ds(side * 1024 + half * 512, 512)],
                            start=(k == 0),
                            stop=(k == LT - 1),
                        )

            def emit_pa(s, ex):
                _, k = divmod(s, LT)
                if k == 0:
                    pa_cur[0] = pap.tile([128, 2048], bf16, tag="pa", name="pa")
                    nc.vector.tensor_copy(pa_cur[0], ex)
                else:
                    nc.vector.tensor_add(pa_cur[0], pa_cur[0], ex)

            def emit_norm(blk):
                # denominator: col-tiled ones-matmul partition-reduce of
                # the partial sums; M=64 replication doubles as the
                # broadcast. Then reciprocal + normalize straight out of
                # the av PSUM into outTP.
                pair, qs = BLOCKS[blk]
                pa = pa_cur[0]
                av = av_cur[0]
                for half in range(2):
                    den = psA2.tile([128, 512], f32, tag="ps2", name=f"den{half}")
                    for side in range(2):
                        nc.tensor.matmul(
                            den[side * HD : (side + 1) * HD, :],
                            ones_sb,
                            pa[:, ds(side * 1024 + half * 512, 512)],
                            start=True,
                            stop=True,
                        )
                    rc = rcp.tile([128, 512], f32, tag="rc", name="rc")
                    nc.vector.reciprocal_approx_fast(rc, den)
                    nc.vector.tensor_mul(
                        outTP_sb[:, pair, ds(qs * 1024 + half * 512, 512)],
                        av[half],
                        rc,
                    )

            # steady state; program order per engine schedules the
            # overlap, the psS/psB pool rotations pace the PE.
            pending_norm = []
            sides_next = emit_scores(0)
            for s in range(NSTEPS):
                blk, k = divmod(s, LT)
                while pending_norm:
                    emit_norm(pending_norm.pop(0))
                pop_due(s)
                ex = emit_exp(sides_next)
                if s + 1 < NSTEPS:
                    sides_next = emit_scores(s + 1)
                emit_av(s, ex)
                emit_pa(s, ex)
                if k == LT - 1:
                    pending_norm.append(blk)
                if s == 2 * LT:  # blocks 0+1 normalized: queue fc lcp=0
                    for i, (et, half) in enumerate(
                        (et, half) for et in range(ET) for half in range(2)
                    ):
                        fillers.append((s + 1 + i, f_fcchunk(0, et, half, "v")))
            while pending_norm:
                emit_norm(pending_norm.pop(0))
            pop_due(NSTEPS + ET * 2)

            # a few warm matmuls to carry the PE through the final
            # normalize window at full clock
            warm2 = psS.tile([128, 512], f32, tag="sc", name="warm2")
            for _ in range(8):
                nc.tensor.matmul(
                    warm2,
                    wo_sb[:, 0, 0, :],
                    outTP_sb[:, 0, 0:512],
                    start=True,
                    stop=True,
                )

            _psB_stack.close()

            # ================= fc tail: lcp=1 =========================
            eng = ["s", "v"]
            for et in range(ET):
                for half in range(2):
                    f_fcchunk(1, et, half, eng[(et * 2 + half) % 2])()

            _psA2_stack.close()
            _psS_stack.close()

    nc.compile()
    return nc


def get_nc():
    global _nc_cache
    if _nc_cache is None:
        _nc_cache = build_nc()
    return _nc_cache


def make_core_inputs(x, Wq, Wk, Wv, Wo, bo):
    """Build the 8 per-core input maps from the full-size inputs."""
    x = np.asarray(x, F32)
    Wq = np.asarray(Wq, F32)
    Wk = np.asarray(Wk, F32)
    Wv = np.asarray(Wv, F32)
    Wo = np.asarray(Wo, F32)
    bo = np.asarray(bo, F32)

    xT_b = [np.ascontiguousarray(x[n].T).astype(BF16) for n in range(NB)]

    in_maps = []
    for c in range(NCORES):
        n, g = divmod(c, HPG)
        heads = [g * HPG + i for i in range(HPG)]

        wqk = np.empty((4, EMBED, 128), F32)
        for j in range(4):
            pair, qk = divmod(j, 2)
            hA = heads[2 * pair]
            hB = heads[2 * pair + 1]
            W = Wq if qk == 0 else Wk
            wqk[j, :, 0:HD] = W[hA * HD : (hA + 1) * HD, :].T
            wqk[j, :, HD:128] = W[hB * HD : (hB + 1) * HD, :].T

        wv = np.concatenate(
            [Wv[h * HD : (h + 1) * HD, :].T for h in heads], axis=1
        )  # [1024, 256]

        wo = np.empty((NPAIRS, ET, 128, 128), F32)
        for pair in range(NPAIRS):
            hA = heads[2 * pair]
            hB = heads[2 * pair + 1]
            for et in range(ET):
                blk = Wo[et * 128 : (et + 1) * 128, :]
                wo[pair, et, 0:HD, :] = blk[:, hA * HD : (hA + 1) * HD].T
                wo[pair, et, HD:128, :] = blk[:, hB * HD : (hB + 1) * HD].T

        # partition-major relayouts: [p, ...] with contiguous per-p rows
        wqk_t = np.ascontiguousarray(
            wqk.reshape(4, ET, 128, 128).transpose(2, 0, 1, 3)
        )
        wv_t = np.ascontiguousarray(
            wv.reshape(ET, 128, HPG * HD).transpose(1, 0, 2)
        )
        wo_t = np.ascontiguousarray(wo.transpose(2, 0, 1, 3))

        in_maps.append(
            {
                "xT": xT_b[n],
                "wqk": wqk_t.astype(BF16),
                "wv": wv_t.astype(BF16),
                "wo": wo_t.astype(BF16),
            }
        )
    return in_maps


def combine_outputs(results, bo):
    """Sum the per-core fc_out partials, add bias, transpose to [N, L, E]."""
    out = np.empty((NB, L, EMBED), F32)
    for n in range(NB):
        acc = results[n * HPG]["out"].astype(F32)
        for g in range(1, HPG):
            acc = acc + results[n * HPG + g]["out"].astype(F32)
        out[n] = acc.T + np.asarray(bo, F32)
    return out


def kernel(x, Wq, Wk, Wv, Wo, bo):
    global LAST_EXEC_TIME_NS, LAST_RESULTS
    nc = get_nc()
    in_maps = make_core_inputs(x, Wq, Wk, Wv, Wo, bo)
    trace = bool(os.environ.get("KERNEL_TRACE"))
    kw = {}
    if trace:
        kw["trace"] = True
        kw["trace_cores"] = list(range(NCORES))
    res = run_bass_kernel_spmd(nc, in_maps, list(range(NCORES)), **kw)
    LAST_EXEC_TIME_NS = res.exec_time_ns
    LAST_RESULTS = res
    return combine_outputs(res.results, bo)
